# revision 12
# baseline (speedup 1.0000x reference)
"""Trainium2 Bass kernel for KG-enhanced embedding model (gnn_message_passing).

Computes, for full inputs:
    inputs_embeds = word_embedding[input_ids]                       # [B,S,H] gather
    h   = relu(entity_embeddings @ W1 + b1)                         # [B,E,MLP_HID]
    ent = h @ W2 + b2                                               # [B,E,H]
    out = inputs_embeds + einsum('bes,beh->bsh', entity_mask, ent)  # masked scatter-add

Sharding: data-parallel over batch B=32 -> 4 examples per core on 8 cores.
Weights (W1,b1,W2,b2) and the vocab table are replicated; the gather reads
only the rows each core needs via indirect DMA.

Shapes (hardcoded): V=30522, H=768, B=32, S=512, E=8, KG=100, MH=1000.
"""

import os
import numpy as np
from contextlib import ExitStack

V, H = 30522, 768
B, S, E = 32, 512, 8
KG, MH = 100, 1000
NCORES = 8
BPC = B // NCORES              # examples per core = 4
TOK = BPC * S                  # tokens per core = 2048
NCH = TOK // 128               # 128-token chunks per core = 16
KCH = 8                        # K chunks of 128 for the 1000(+bias) contraction

_PROGRAM = None


def _maybe_enable_profiling():
    """Optional NTFF profiling (KERNEL_PROFILE=1): shim antenv.axon_hooks."""
    if os.environ.get("KERNEL_PROFILE") != "1":
        return False
    import sys, types
    try:
        from antenv.axon_hooks import get_axon_ntff_profile_hook  # noqa: F401
        return True
    except ImportError:
        pass
    try:
        from trn_agent_boot.trn_boot import _ntff_profile_via_ctypes
        import antenv
        hook = _ntff_profile_via_ctypes("/opt/axon/libaxon_pjrt.so")
        m = types.ModuleType("antenv.axon_hooks")
        m.get_axon_ntff_profile_hook = lambda: hook
        m.set_axon_ntff_profile_hook = lambda h: None
        sys.modules["antenv.axon_hooks"] = m
        antenv.axon_hooks = m
        return True
    except Exception:
        return False


def _build_program():
    import concourse.bacc as bacc
    import concourse.tile as tile
    from concourse import bass, mybir

    f32 = mybir.dt.float32
    i32 = mybir.dt.int32
    RELU = mybir.ActivationFunctionType.Relu

    nc = bacc.Bacc("TRN2", target_bir_lowering=False, debug=False)

    ids_ap = nc.dram_tensor("idsT", [128, NCH], i32, kind="ExternalInput").ap()
    we_ap = nc.dram_tensor("we", [V, H], f32, kind="ExternalInput").ap()
    eeT_ap = nc.dram_tensor("eeT1", [KG + 1, BPC * E], f32, kind="ExternalInput").ap()
    w11_ap = nc.dram_tensor("w11", [KG + 1, MH], f32, kind="ExternalInput").ap()
    w21_ap = nc.dram_tensor("w21p", [128, KCH * H], f32, kind="ExternalInput").ap()
    b2_ap = nc.dram_tensor("b2row", [1, H], f32, kind="ExternalInput").ap()
    # block-diagonal mask: row b*E+e covers only example b's token columns
    maskT_ap = nc.dram_tensor("maskT", [BPC * E, TOK], f32, kind="ExternalInput").ap()
    out_ap = nc.dram_tensor("out", [TOK, H], f32, kind="ExternalOutput").ap()

    NE = BPC * E  # entities per core = 32

    with tile.TileContext(nc) as tc, ExitStack() as ctx:
        const = ctx.enter_context(tc.tile_pool(name="const", bufs=1))
        psA = ctx.enter_context(tc.tile_pool(name="psA", bufs=1, space="PSUM"))
        psB = ctx.enter_context(tc.tile_pool(name="psB", bufs=1, space="PSUM"))
        psC = ctx.enter_context(tc.tile_pool(name="psC", bufs=2, space="PSUM"))
        gpool = ctx.enter_context(tc.tile_pool(name="gath", bufs=4))
        opool = ctx.enter_context(tc.tile_pool(name="outp", bufs=4))

        # ---- constant loads -------------------------------------------------
        ids_sb = const.tile([128, NCH], i32)
        nc.sync.dma_start(ids_sb[:], ids_ap[:])
        eeT_sb = const.tile([KG + 1, NE], f32)
        nc.sync.dma_start(eeT_sb[:], eeT_ap[:])
        w11_sb = const.tile([KG + 1, MH], f32)
        nc.sync.dma_start(w11_sb[:], w11_ap[:])
        w21_sb = const.tile([128, KCH * H], f32)
        nc.sync.dma_start(w21_sb[:], w21_ap[:])
        maskT_sb = const.tile([BPC * E, TOK], f32)
        nc.sync.dma_start(maskT_sb[:], maskT_ap[:])
        b2_sb = const.tile([1, H], f32)
        nc.sync.dma_start(b2_sb[:], b2_ap[:])
        ones1 = const.tile([1, NE], f32)
        nc.vector.memset(ones1[:], 1.0)

        # ---- MLP stage 1: hT[k*128+p, e] = relu(W1.T @ ee.T + b1) ----------
        # Contraction K=101 (KG rows + ones row for b1). Output kept
        # transposed in K-chunks of 128 so stage 2 needs no transpose.
        hT = const.tile([128, KCH, NE], f32)
        # zero the tail of the last K-chunk before relu fills rows 0..103
        # (w21p rows past MH are zero, but garbage SBUF could be NaN: NaN*0=NaN)
        nc.vector.memset(hT[96:128, KCH - 1, :], 0.0)
        for k in range(KCH):
            mw = 128 if k < KCH - 1 else MH - 128 * (KCH - 1)  # 104 in last chunk
            ps = psA.tile([128, NE], f32)
            nc.tensor.matmul(
                out=ps[:mw, :],
                lhsT=w11_sb[:, k * 128 : k * 128 + mw],
                rhs=eeT_sb[:],
                start=True,
                stop=True,
            )
            nc.scalar.activation(out=hT[:mw, k, :], in_=ps[:mw, :], func=RELU)

        # ---- MLP stage 2: ent[e, h] = hT.T @ W2 + b2 ------------------------
        # b2 enters the PSUM accumulation as a K=1 matmul: ones[1,NE].T @ b2[1,H]
        entp = psB.tile([NE, H], f32)
        for n0, n1 in ((0, 512), (512, H)):
            for k in range(KCH):
                nc.tensor.matmul(
                    out=entp[:, n0:n1],
                    lhsT=hT[:, k, :],
                    rhs=w21_sb[:, k * H + n0 : k * H + n1],
                    start=(k == 0),
                    stop=False,
                )
            nc.tensor.matmul(
                out=entp[:, n0:n1],
                lhsT=ones1[:],
                rhs=b2_sb[:, n0:n1],
                start=False,
                stop=True,
            )
        ent = const.tile([NE, H], f32)
        nc.scalar.copy(ent[:], entp[:])

        # ---- main loop: gather 128 rows, scatter-matmul, add, store --------
        for g in range(NCH):
            gt = gpool.tile([128, H], f32)
            nc.gpsimd.indirect_dma_start(
                out=gt[:],
                out_offset=None,
                in_=we_ap[:],
                in_offset=bass.IndirectOffsetOnAxis(ap=ids_sb[:, g : g + 1], axis=0),
            )
            sc = psC.tile([128, H], f32)
            for n0, n1 in ((0, 512), (512, H)):
                nc.tensor.matmul(
                    out=sc[:, n0:n1],
                    lhsT=maskT_sb[:, g * 128 : (g + 1) * 128],
                    rhs=ent[:, n0:n1],
                    start=True,
                    stop=True,
                )
            ot = opool.tile([128, H], f32)
            nc.vector.tensor_add(ot[:], gt[:], sc[:])
            nc.sync.dma_start(out_ap[g * 128 : (g + 1) * 128, :], ot[:])

    nc.compile()
    return nc


def _get_program():
    global _PROGRAM
    if _PROGRAM is None:
        _PROGRAM = _build_program()
    return _PROGRAM


def _prep_shards(inputs):
    ids = np.ascontiguousarray(np.asarray(inputs["input_ids"]).astype(np.int32))
    ee = np.asarray(inputs["entity_embeddings"], dtype=np.float32)
    mask = np.asarray(inputs["entity_mask"], dtype=np.float32)
    we = np.ascontiguousarray(np.asarray(inputs["word_embedding"], dtype=np.float32))
    W1 = np.asarray(inputs["W1"], dtype=np.float32)
    b1 = np.asarray(inputs["b1"], dtype=np.float32)
    W2 = np.asarray(inputs["W2"], dtype=np.float32)
    b2 = np.asarray(inputs["b2"], dtype=np.float32)

    w11 = np.ascontiguousarray(np.concatenate([W1, b1[None, :]], 0))  # [101, MH]
    w21_pad = np.concatenate(
        [W2, np.zeros((KCH * 128 - MH, H), np.float32)], 0
    )  # [1024, H]
    w21p = np.ascontiguousarray(
        w21_pad.reshape(KCH, 128, H).transpose(1, 0, 2).reshape(128, KCH * H)
    )

    in_maps = []
    for i in range(NCORES):
        sl = slice(BPC * i, BPC * (i + 1))
        ids_i = ids[sl].reshape(-1)  # [TOK]
        idsT = np.ascontiguousarray(ids_i.reshape(NCH, 128).T)  # [128, NCH]
        eeT1 = np.ascontiguousarray(
            np.concatenate(
                [ee[sl].reshape(BPC * E, KG).T, np.ones((1, BPC * E), np.float32)], 0
            )
        )  # [101, 32]
        # block-diagonal [BPC*E, TOK]: row b*E+e nonzero only in example b's cols
        maskT = np.zeros((BPC * E, TOK), np.float32)
        for b in range(BPC):
            maskT[b * E : (b + 1) * E, b * S : (b + 1) * S] = mask[BPC * i + b]
        in_maps.append(
            {
                "idsT": idsT,
                "we": we,
                "eeT1": eeT1,
                "w11": w11,
                "w21p": w21p,
                "b2row": np.ascontiguousarray(b2[None, :]),
                "maskT": maskT,
            }
        )
    return in_maps


def kernel(**inputs) -> np.ndarray:
    from concourse.bass_utils import run_bass_kernel_spmd

    trace = _maybe_enable_profiling()
    nc = _get_program()
    in_maps = _prep_shards(inputs)
    res = run_bass_kernel_spmd(
        nc, in_maps, core_ids=list(range(NCORES)), trace=trace
    )
    if trace and res.exec_time_ns is not None:
        print(f"HW exec time: {res.exec_time_ns} ns")
    out = np.concatenate(
        [res.results[i]["out"].reshape(BPC, S, H) for i in range(NCORES)], 0
    )
    return out


if __name__ == "__main__":
    rng = np.random.default_rng(0)
    inputs = {
        "input_ids": rng.integers(0, V, (B, S)).astype(np.int32),
        "entity_embeddings": rng.standard_normal((B, E, KG), dtype=np.float32),
        "entity_mask": (rng.random((B, E, S)) < 0.02).astype(np.float32),
        "word_embedding": rng.standard_normal((V, H), dtype=np.float32) * 0.02,
        "W1": rng.standard_normal((KG, MH), dtype=np.float32) * 0.02,
        "b1": np.zeros(MH, np.float32),
        "W2": rng.standard_normal((MH, H), dtype=np.float32) * 0.02,
        "b2": np.zeros(H, np.float32),
    }
    out = kernel(**inputs)
    ref = inputs["word_embedding"][inputs["input_ids"]] + np.einsum(
        "bes,beh->bsh",
        inputs["entity_mask"],
        np.maximum(
            inputs["entity_embeddings"] @ inputs["W1"] + inputs["b1"], 0.0
        )
        @ inputs["W2"]
        + inputs["b2"],
    )
    err = np.abs(out - ref).max() / max(np.abs(ref).max(), 1e-12)
    print("self-check rel err:", err)


# revision 13
# speedup vs baseline: 1.1110x; 1.1110x over previous
"""Trainium2 Bass kernel for KG-enhanced embedding model (gnn_message_passing).

Computes, for full inputs:
    inputs_embeds = word_embedding[input_ids]                       # [B,S,H] gather
    h   = relu(entity_embeddings @ W1 + b1)                         # [B,E,MLP_HID]
    ent = h @ W2 + b2                                               # [B,E,H]
    out = inputs_embeds + einsum('bes,beh->bsh', entity_mask, ent)  # masked scatter-add

Sharding: data-parallel over batch B=32 -> 4 examples per core on 8 cores.
Weights and the vocab table are replicated; the gather reads only the rows
each core needs via indirect DMA.

All matmuls run in bf16 with hi/lo split precision (x = hi + lo, both bf16)
so products carry ~2^-17 relative error instead of bf16's 2^-9, while
streaming through the PE at the bf16 rate (fp32 matmul is ~4-5x slower).
The mask is 0/1 so it is exact in bf16.

Shapes (hardcoded): V=30522, H=768, B=32, S=512, E=8, KG=100, MH=1000.
"""

import os
import numpy as np
from contextlib import ExitStack

V, H = 30522, 768
B, S, E = 32, 512, 8
KG, MH = 100, 1000
NCORES = 8
BPC = B // NCORES              # examples per core = 4
TOK = BPC * S                  # tokens per core = 2048
NCH = TOK // 128               # 128-token chunks per core = 16
KCH = 8                        # K chunks of 128 for the 1000-dim contraction
NE = BPC * E                   # entities per core = 32

_PROGRAM = None


def _maybe_enable_profiling():
    """Optional NTFF profiling (KERNEL_PROFILE=1): shim antenv.axon_hooks."""
    if os.environ.get("KERNEL_PROFILE") != "1":
        return False
    import sys, types
    try:
        from antenv.axon_hooks import get_axon_ntff_profile_hook  # noqa: F401
        return True
    except ImportError:
        pass
    try:
        from trn_agent_boot.trn_boot import _ntff_profile_via_ctypes
        import antenv
        hook = _ntff_profile_via_ctypes("/opt/axon/libaxon_pjrt.so")
        m = types.ModuleType("antenv.axon_hooks")
        m.get_axon_ntff_profile_hook = lambda: hook
        m.set_axon_ntff_profile_hook = lambda h: None
        sys.modules["antenv.axon_hooks"] = m
        antenv.axon_hooks = m
        return True
    except Exception:
        return False


def _build_program():
    import concourse.bacc as bacc
    import concourse.tile as tile
    from concourse import bass, mybir

    f32 = mybir.dt.float32
    bf16 = mybir.dt.bfloat16
    i32 = mybir.dt.int32
    RELU = mybir.ActivationFunctionType.Relu
    SUB = mybir.AluOpType.subtract

    nc = bacc.Bacc("TRN2", target_bir_lowering=False, debug=False)

    ids_ap = nc.dram_tensor("idsT", [128, NCH], i32, kind="ExternalInput").ap()
    we_ap = nc.dram_tensor("we", [V, H], f32, kind="ExternalInput").ap()
    eeT_h_ap = nc.dram_tensor("eeT_hi", [KG, NE], bf16, kind="ExternalInput").ap()
    eeT_l_ap = nc.dram_tensor("eeT_lo", [KG, NE], bf16, kind="ExternalInput").ap()
    w1_h_ap = nc.dram_tensor("w1_hi", [KG, MH], bf16, kind="ExternalInput").ap()
    w1_l_ap = nc.dram_tensor("w1_lo", [KG, MH], bf16, kind="ExternalInput").ap()
    b1_ap = nc.dram_tensor("b1row", [1, MH], f32, kind="ExternalInput").ap()
    w2_h_ap = nc.dram_tensor("w2p_hi", [128, KCH * H], bf16, kind="ExternalInput").ap()
    w2_l_ap = nc.dram_tensor("w2p_lo", [128, KCH * H], bf16, kind="ExternalInput").ap()
    b2_ap = nc.dram_tensor("b2hl", [2, H], bf16, kind="ExternalInput").ap()
    maskT_ap = nc.dram_tensor("maskT2", [2 * NE, TOK], bf16, kind="ExternalInput").ap()
    out_ap = nc.dram_tensor("out", [TOK, H], f32, kind="ExternalOutput").ap()

    with tile.TileContext(nc) as tc, ExitStack() as ctx:
        const = ctx.enter_context(tc.tile_pool(name="const", bufs=1))
        psA = ctx.enter_context(tc.tile_pool(name="psA", bufs=1, space="PSUM"))
        psB = ctx.enter_context(tc.tile_pool(name="psB", bufs=1, space="PSUM"))
        psC = ctx.enter_context(tc.tile_pool(name="psC", bufs=2, space="PSUM"))
        gpool = ctx.enter_context(tc.tile_pool(name="gath", bufs=4))
        opool = ctx.enter_context(tc.tile_pool(name="outp", bufs=4))

        # ---- constant loads -------------------------------------------------
        ids_sb = const.tile([128, NCH], i32)
        nc.sync.dma_start(ids_sb[:], ids_ap[:])
        eeT_hi = const.tile([KG, NE], bf16)
        nc.sync.dma_start(eeT_hi[:], eeT_h_ap[:])
        eeT_lo = const.tile([KG, NE], bf16)
        nc.sync.dma_start(eeT_lo[:], eeT_l_ap[:])
        w1_hi = const.tile([KG, MH], bf16)
        nc.sync.dma_start(w1_hi[:], w1_h_ap[:])
        w1_lo = const.tile([KG, MH], bf16)
        nc.sync.dma_start(w1_lo[:], w1_l_ap[:])
        b1_sb = const.tile([1, MH], f32)
        nc.sync.dma_start(b1_sb[:], b1_ap[:])
        w2_hi = const.tile([128, KCH * H], bf16)
        nc.sync.dma_start(w2_hi[:], w2_h_ap[:])
        w2_lo = const.tile([128, KCH * H], bf16)
        nc.sync.dma_start(w2_lo[:], w2_l_ap[:])
        b2_sb = const.tile([2, H], bf16)
        nc.sync.dma_start(b2_sb[:], b2_ap[:])
        maskT_sb = const.tile([2 * NE, TOK], bf16)
        nc.sync.dma_start(maskT_sb[:], maskT_ap[:])
        ones2 = const.tile([2, NE], bf16)
        nc.vector.memset(ones2[:], 1.0)

        # ---- MLP stage 1: hT[k*128+p, e] = relu(W1.T @ ee.T + b1) ----------
        # 3-term split product: W1hi*eehi + W1hi*eelo + W1lo*eehi.
        # b1 is added via the activation bias port (per-partition scalar).
        hT_hi = const.tile([128, KCH, NE], bf16)
        hT_lo = const.tile([128, KCH, NE], bf16)
        hF = const.tile([128, KCH, NE], f32)
        nc.vector.memset(hT_hi[96:128, KCH - 1, :], 0.0)
        nc.vector.memset(hT_lo[96:128, KCH - 1, :], 0.0)
        # b1 as a [MH] column: partition p of chunk k holds b1[k*128+p].
        # Load b1 transposed via per-chunk [mw,1] slices of the [1,MH] row.
        b1_col = const.tile([128, KCH], f32)
        for k in range(KCH):
            mw = 128 if k < KCH - 1 else MH - 128 * (KCH - 1)  # 104 in last
            nc.sync.dma_start(
                b1_col[:mw, k : k + 1], b1_ap[0, k * 128 : k * 128 + mw, None]
            )
        for k in range(KCH):
            mw = 128 if k < KCH - 1 else MH - 128 * (KCH - 1)
            ps = psA.tile([128, NE], f32)
            for term, (lt, rt) in enumerate(
                ((w1_hi, eeT_hi), (w1_hi, eeT_lo), (w1_lo, eeT_hi))
            ):
                nc.tensor.matmul(
                    out=ps[:mw, :],
                    lhsT=lt[:, k * 128 : k * 128 + mw],
                    rhs=rt[:],
                    start=(term == 0),
                    stop=(term == 2),
                )
            # h = relu(ps + b1); keep f32 then split into bf16 hi/lo
            nc.scalar.activation(
                out=hF[:mw, k, :],
                in_=ps[:mw, :],
                func=RELU,
                bias=b1_col[:mw, k : k + 1],
            )
            nc.vector.tensor_copy(out=hT_hi[:mw, k, :], in_=hF[:mw, k, :])
            nc.vector.tensor_tensor(
                out=hT_lo[:mw, k, :],
                in0=hF[:mw, k, :],
                in1=hT_hi[:mw, k, :],
                op=SUB,
            )

        # ---- MLP stage 2: ent = hT.T @ W2 + b2, split into entHL ------------
        # 3-term: hhi*W2hi + hlo*W2hi + hhi*W2lo; b2 enters as a K=2 matmul
        # of ones2.T @ [b2hi; b2lo].
        entp = psB.tile([NE, H], f32)
        for n0, n1 in ((0, 512), (512, H)):
            for k in range(KCH):
                for term, (lt, rt) in enumerate(
                    ((hT_hi, w2_hi), (hT_lo, w2_hi), (hT_hi, w2_lo))
                ):
                    nc.tensor.matmul(
                        out=entp[:, n0:n1],
                        lhsT=lt[:, k, :],
                        rhs=rt[:, k * H + n0 : k * H + n1],
                        start=(k == 0 and term == 0),
                        stop=False,
                    )
            nc.tensor.matmul(
                out=entp[:, n0:n1],
                lhsT=ones2[:],
                rhs=b2_sb[:, n0:n1],
                start=False,
                stop=True,
            )
        # split ent into bf16 hi/lo, K-stacked on partitions [hi(0:32), lo(32:64)]
        entHL = const.tile([2 * NE, H], bf16)
        ent_lo_tmp = const.tile([NE, H], bf16)
        nc.scalar.copy(entHL[:NE, :], entp[:])  # cast f32 -> bf16 (hi)
        nc.vector.tensor_tensor(
            out=ent_lo_tmp[:], in0=entp[:], in1=entHL[:NE, :], op=SUB
        )
        # partition shift 0:32 -> 32:64 needs a DMA (engines can't cross lanes)
        nc.sync.dma_start(entHL[NE : 2 * NE, :], ent_lo_tmp[:])

        # ---- main loop: gather 128 rows, scatter-matmul, add, store --------
        for g in range(NCH):
            gt = gpool.tile([128, H], f32)
            nc.gpsimd.indirect_dma_start(
                out=gt[:],
                out_offset=None,
                in_=we_ap[:],
                in_offset=bass.IndirectOffsetOnAxis(ap=ids_sb[:, g : g + 1], axis=0),
            )
            sc = psC.tile([128, H], f32)
            for n0, n1 in ((0, 512), (512, H)):
                nc.tensor.matmul(
                    out=sc[:, n0:n1],
                    lhsT=maskT_sb[:, g * 128 : (g + 1) * 128],
                    rhs=entHL[:, n0:n1],
                    start=True,
                    stop=True,
                )
            ot = opool.tile([128, H], f32)
            nc.vector.tensor_add(ot[:], gt[:], sc[:])
            eng = nc.sync if g % 2 == 0 else nc.scalar
            eng.dma_start(out_ap[g * 128 : (g + 1) * 128, :], ot[:])

    nc.compile()
    return nc


def _get_program():
    global _PROGRAM
    if _PROGRAM is None:
        _PROGRAM = _build_program()
    return _PROGRAM


def _split_hl(x):
    import ml_dtypes

    hi = x.astype(ml_dtypes.bfloat16)
    lo = (x - hi.astype(np.float32)).astype(ml_dtypes.bfloat16)
    return np.ascontiguousarray(hi), np.ascontiguousarray(lo)


def _prep_shards(inputs):
    import ml_dtypes

    ids = np.ascontiguousarray(np.asarray(inputs["input_ids"]).astype(np.int32))
    ee = np.asarray(inputs["entity_embeddings"], dtype=np.float32)
    mask = np.asarray(inputs["entity_mask"], dtype=np.float32)
    we = np.ascontiguousarray(np.asarray(inputs["word_embedding"], dtype=np.float32))
    W1 = np.asarray(inputs["W1"], dtype=np.float32)
    b1 = np.asarray(inputs["b1"], dtype=np.float32)
    W2 = np.asarray(inputs["W2"], dtype=np.float32)
    b2 = np.asarray(inputs["b2"], dtype=np.float32)

    w1_hi, w1_lo = _split_hl(W1)  # [KG, MH]
    w2_pad = np.concatenate([W2, np.zeros((KCH * 128 - MH, H), np.float32)], 0)
    w2p = w2_pad.reshape(KCH, 128, H).transpose(1, 0, 2).reshape(128, KCH * H)
    w2p_hi, w2p_lo = _split_hl(w2p)
    b2_hi, b2_lo = _split_hl(b2[None, :])
    b2hl = np.ascontiguousarray(np.concatenate([b2_hi, b2_lo], 0))  # [2, H]

    in_maps = []
    for i in range(NCORES):
        sl = slice(BPC * i, BPC * (i + 1))
        ids_i = ids[sl].reshape(-1)  # [TOK]
        idsT = np.ascontiguousarray(ids_i.reshape(NCH, 128).T)  # [128, NCH]
        eeT = ee[sl].reshape(NE, KG).T  # [KG, NE]
        eeT_hi, eeT_lo = _split_hl(eeT)
        # block-diagonal [NE, TOK] mask, duplicated to [2*NE, TOK] so one
        # K=64 matmul covers both the hi and lo halves of entHL
        maskT = np.zeros((NE, TOK), np.float32)
        for b in range(BPC):
            maskT[b * E : (b + 1) * E, b * S : (b + 1) * S] = mask[BPC * i + b]
        maskT2 = np.ascontiguousarray(
            np.concatenate([maskT, maskT], 0).astype(ml_dtypes.bfloat16)
        )
        in_maps.append(
            {
                "idsT": idsT,
                "we": we,
                "eeT_hi": eeT_hi,
                "eeT_lo": eeT_lo,
                "w1_hi": w1_hi,
                "w1_lo": w1_lo,
                "b1row": np.ascontiguousarray(b1[None, :]),
                "w2p_hi": w2p_hi,
                "w2p_lo": w2p_lo,
                "b2hl": b2hl,
                "maskT2": maskT2,
            }
        )
    return in_maps


def kernel(**inputs) -> np.ndarray:
    from concourse.bass_utils import run_bass_kernel_spmd

    trace = _maybe_enable_profiling()
    nc = _get_program()
    in_maps = _prep_shards(inputs)
    res = run_bass_kernel_spmd(
        nc, in_maps, core_ids=list(range(NCORES)), trace=trace
    )
    if trace and res.exec_time_ns is not None:
        print(f"HW exec time: {res.exec_time_ns} ns")
    out = np.concatenate(
        [res.results[i]["out"].reshape(BPC, S, H) for i in range(NCORES)], 0
    )
    return out


if __name__ == "__main__":
    rng = np.random.default_rng(0)
    inputs = {
        "input_ids": rng.integers(0, V, (B, S)).astype(np.int32),
        "entity_embeddings": rng.standard_normal((B, E, KG), dtype=np.float32),
        "entity_mask": (rng.random((B, E, S)) < 0.02).astype(np.float32),
        "word_embedding": rng.standard_normal((V, H), dtype=np.float32) * 0.02,
        "W1": rng.standard_normal((KG, MH), dtype=np.float32) * 0.02,
        "b1": np.zeros(MH, np.float32),
        "W2": rng.standard_normal((MH, H), dtype=np.float32) * 0.02,
        "b2": np.zeros(H, np.float32),
    }
    out = kernel(**inputs)
    ref = inputs["word_embedding"][inputs["input_ids"]] + np.einsum(
        "bes,beh->bsh",
        inputs["entity_mask"],
        np.maximum(
            inputs["entity_embeddings"] @ inputs["W1"] + inputs["b1"], 0.0
        )
        @ inputs["W2"]
        + inputs["b2"],
    )
    err = np.abs(out - ref).max() / max(np.abs(ref).max(), 1e-12)
    print("self-check rel err:", err)


# revision 21
# speedup vs baseline: 1.3740x; 1.2367x over previous
"""Trainium2 Bass kernel for KG-enhanced embedding model (gnn_message_passing).

Computes, for full inputs:
    inputs_embeds = word_embedding[input_ids]                       # [B,S,H] gather
    h   = relu(entity_embeddings @ W1 + b1)                         # [B,E,MLP_HID]
    ent = h @ W2 + b2                                               # [B,E,H]
    out = inputs_embeds + einsum('bes,beh->bsh', entity_mask, ent)  # masked scatter-add

Sharding: data-parallel over batch B=32 -> 4 examples per core on 8 cores.
Weights and the vocab table are replicated; the gather reads only the rows
each core needs via indirect DMA.

All matmuls run in bf16 with hi/lo split precision (x = hi + lo, both bf16)
so products carry ~2^-17 relative error instead of bf16's 2^-9, while
streaming through the PE at the bf16 rate (fp32 matmul is ~4-5x slower).
The mask is 0/1 so it is exact in bf16.

Shapes (hardcoded): V=30522, H=768, B=32, S=512, E=8, KG=100, MH=1000.
"""

import os
import numpy as np
from contextlib import ExitStack

V, H = 30522, 768
B, S, E = 32, 512, 8
KG, MH = 100, 1000
NCORES = 8
BPC = B // NCORES              # examples per core = 4
TOK = BPC * S                  # tokens per core = 2048
NCH = TOK // 128               # 128-token chunks per core = 16
KCH = 8                        # K chunks of 128 for the 1000-dim contraction
NE = BPC * E                   # entities per core = 32

_PROGRAM = None


def _maybe_enable_profiling():
    """Optional NTFF profiling (KERNEL_PROFILE=1): shim antenv.axon_hooks."""
    if os.environ.get("KERNEL_PROFILE") != "1":
        return False
    import sys, types
    try:
        from antenv.axon_hooks import get_axon_ntff_profile_hook  # noqa: F401
        return True
    except ImportError:
        pass
    try:
        from trn_agent_boot.trn_boot import _ntff_profile_via_ctypes
        import antenv
        hook = _ntff_profile_via_ctypes("/opt/axon/libaxon_pjrt.so")
        m = types.ModuleType("antenv.axon_hooks")
        m.get_axon_ntff_profile_hook = lambda: hook
        m.set_axon_ntff_profile_hook = lambda h: None
        sys.modules["antenv.axon_hooks"] = m
        antenv.axon_hooks = m
        return True
    except Exception:
        return False


def _build_program():
    import concourse.bacc as bacc
    import concourse.tile as tile
    from concourse import bass, mybir

    f32 = mybir.dt.float32
    bf16 = mybir.dt.bfloat16
    i32 = mybir.dt.int32
    RELU = mybir.ActivationFunctionType.Relu
    SUB = mybir.AluOpType.subtract

    nc = bacc.Bacc("TRN2", target_bir_lowering=False, debug=False)

    ids_ap = nc.dram_tensor("idsT", [128, NCH], i32, kind="ExternalInput").ap()
    we_ap = nc.dram_tensor("we", [V, H], f32, kind="ExternalInput").ap()
    eeT_h_ap = nc.dram_tensor("eeT_hi", [KG, NE], bf16, kind="ExternalInput").ap()
    eeT_l_ap = nc.dram_tensor("eeT_lo", [KG, NE], bf16, kind="ExternalInput").ap()
    w1_h_ap = nc.dram_tensor("w1_hi", [KG, MH], bf16, kind="ExternalInput").ap()
    w1_l_ap = nc.dram_tensor("w1_lo", [KG, MH], bf16, kind="ExternalInput").ap()
    b1_ap = nc.dram_tensor("b1row", [1, MH], f32, kind="ExternalInput").ap()
    w2_h_ap = nc.dram_tensor("w2p_hi", [128, KCH * H], bf16, kind="ExternalInput").ap()
    w2_l_ap = nc.dram_tensor("w2p_lo", [128, KCH * H], bf16, kind="ExternalInput").ap()
    b1c_ap = nc.dram_tensor("b1colT", [128, KCH], f32, kind="ExternalInput").ap()
    b2_ap = nc.dram_tensor("b2hl", [2, H], bf16, kind="ExternalInput").ap()
    maskT_ap = nc.dram_tensor("maskT2", [2 * NE, TOK], bf16, kind="ExternalInput").ap()
    out_ap = nc.dram_tensor("out", [TOK, H], f32, kind="ExternalOutput").ap()

    with tile.TileContext(nc) as tc, ExitStack() as ctx:
        const = ctx.enter_context(tc.tile_pool(name="const", bufs=1))
        psA = ctx.enter_context(tc.tile_pool(name="psA", bufs=2, space="PSUM"))
        psB = ctx.enter_context(tc.tile_pool(name="psB", bufs=1, space="PSUM"))
        psC = ctx.enter_context(tc.tile_pool(name="psC", bufs=2, space="PSUM"))
        gpool = ctx.enter_context(tc.tile_pool(name="gath", bufs=NCH))
        opool = ctx.enter_context(tc.tile_pool(name="outp", bufs=6))

        # ---- index load + all gathers issued first --------------------------
        # (gathers only depend on ids; with NCH buffers they all run while
        # the MLP phase is still loading weights / computing)
        ids_sb = const.tile([128, NCH], i32)
        nc.sync.dma_start(ids_sb[:], ids_ap[:])
        gts = []
        for g in range(NCH):
            gt = gpool.tile([128, H], f32)
            nc.gpsimd.indirect_dma_start(
                out=gt[:],
                out_offset=None,
                in_=we_ap[:],
                in_offset=bass.IndirectOffsetOnAxis(ap=ids_sb[:, g : g + 1], axis=0),
            )
            gts.append(gt)

        # ---- constant loads, small-first so mm1 starts early ---------------
        eeT_hi = const.tile([KG, NE], bf16)
        nc.sync.dma_start(eeT_hi[:], eeT_h_ap[:])
        eeT_lo = const.tile([KG, NE], bf16)
        nc.sync.dma_start(eeT_lo[:], eeT_l_ap[:])
        b1_col = const.tile([128, KCH], f32)
        nc.sync.dma_start(b1_col[:], b1c_ap[:])
        b2_sb = const.tile([2, H], bf16)
        nc.sync.dma_start(b2_sb[:], b2_ap[:])
        w1_hi = const.tile([KG, MH], bf16)
        nc.sync.dma_start(w1_hi[:], w1_h_ap[:])
        w1_lo = const.tile([KG, MH], bf16)
        nc.sync.dma_start(w1_lo[:], w1_l_ap[:])
        maskT_sb = const.tile([2 * NE, TOK], bf16)
        nc.sync.dma_start(maskT_sb[:], maskT_ap[:])
        # w2 chunk-wise so mm2's k-th step starts as soon as its chunk lands
        w2_hi = const.tile([128, KCH * H], bf16)
        w2_lo = const.tile([128, KCH * H], bf16)
        for k in range(KCH):
            csl = slice(k * H, (k + 1) * H)
            nc.sync.dma_start(w2_hi[:, csl], w2_h_ap[:, csl])
            nc.sync.dma_start(w2_lo[:, csl], w2_l_ap[:, csl])
        ones2 = const.tile([2, NE], bf16)
        nc.vector.memset(ones2[:], 1.0)

        # ---- MLP stage 1: hT[k*128+p, e] = relu(W1.T @ ee.T + b1) ----------
        # 3-term split product: W1hi*eehi + W1hi*eelo + W1lo*eehi.
        # b1 is added via the activation bias port (per-partition scalar).
        hT_hi = const.tile([128, KCH, NE], bf16)
        hT_lo = const.tile([128, KCH, NE], bf16)
        hF = const.tile([128, KCH, NE], f32)
        nc.vector.memset(hT_hi[96:128, KCH - 1, :], 0.0)
        nc.vector.memset(hT_lo[96:128, KCH - 1, :], 0.0)
        for k in range(KCH):
            mw = 128 if k < KCH - 1 else MH - 128 * (KCH - 1)
            ps = psA.tile([128, NE], f32)
            for term, (lt, rt) in enumerate(
                ((w1_hi, eeT_hi), (w1_hi, eeT_lo), (w1_lo, eeT_hi))
            ):
                nc.tensor.matmul(
                    out=ps[:mw, :],
                    lhsT=lt[:, k * 128 : k * 128 + mw],
                    rhs=rt[:],
                    start=(term == 0),
                    stop=(term == 2),
                )
            # h = relu(ps + b1); keep f32 then split into bf16 hi/lo
            nc.scalar.activation(
                out=hF[:mw, k, :],
                in_=ps[:mw, :],
                func=RELU,
                bias=b1_col[:mw, k : k + 1],
            )
            nc.vector.tensor_copy(out=hT_hi[:mw, k, :], in_=hF[:mw, k, :])
            nc.vector.tensor_tensor(
                out=hT_lo[:mw, k, :],
                in0=hF[:mw, k, :],
                in1=hT_hi[:mw, k, :],
                op=SUB,
            )

        # ---- MLP stage 2: ent = hT.T @ W2 + b2, split into entHL ------------
        # 3-term: hhi*W2hi + hlo*W2hi + hhi*W2lo; b2 enters as a K=2 matmul
        # of ones2.T @ [b2hi; b2lo].
        entp = psB.tile([NE, H], f32)
        for n0, n1 in ((0, 512), (512, H)):
            nc.tensor.matmul(
                out=entp[:, n0:n1],
                lhsT=ones2[:],
                rhs=b2_sb[:, n0:n1],
                start=True,
                stop=False,
            )
            for k in range(KCH):
                for term, (lt, rt) in enumerate(
                    ((hT_hi, w2_hi), (hT_lo, w2_hi), (hT_hi, w2_lo))
                ):
                    nc.tensor.matmul(
                        out=entp[:, n0:n1],
                        lhsT=lt[:, k, :],
                        rhs=rt[:, k * H + n0 : k * H + n1],
                        start=False,
                        stop=(k == KCH - 1 and term == 2),
                    )
        # split ent into bf16 hi/lo, K-stacked on partitions [hi(0:32), lo(32:64)]
        entHL = const.tile([2 * NE, H], bf16)
        ent_lo_tmp = const.tile([NE, H], bf16)
        nc.scalar.copy(entHL[:NE, :], entp[:])  # cast f32 -> bf16 (hi)
        nc.vector.tensor_tensor(
            out=ent_lo_tmp[:], in0=entp[:], in1=entHL[:NE, :], op=SUB
        )
        # partition shift 0:32 -> 32:64 needs a DMA (engines can't cross lanes)
        nc.sync.dma_start(entHL[NE : 2 * NE, :], ent_lo_tmp[:])

        # ---- main loop: scatter-matmul, add (DVE/GpSimd split), store ------
        for g in range(NCH):
            gt = gts[g]
            sc = psC.tile([128, H], f32)
            for n0, n1 in ((0, 512), (512, H)):
                nc.tensor.matmul(
                    out=sc[:, n0:n1],
                    lhsT=maskT_sb[:, g * 128 : (g + 1) * 128],
                    rhs=entHL[:, n0:n1],
                    start=True,
                    stop=True,
                )
            ot = opool.tile([128, H], f32)
            nc.vector.tensor_add(ot[:], gt[:], sc[:])
            st_eng = nc.sync if g % 2 == 0 else nc.scalar
            st_eng.dma_start(out_ap[g * 128 : (g + 1) * 128, :], ot[:])

    nc.compile()
    return nc


def _get_program():
    global _PROGRAM
    if _PROGRAM is None:
        _PROGRAM = _build_program()
    return _PROGRAM


def _split_hl(x):
    import ml_dtypes

    hi = x.astype(ml_dtypes.bfloat16)
    lo = (x - hi.astype(np.float32)).astype(ml_dtypes.bfloat16)
    return np.ascontiguousarray(hi), np.ascontiguousarray(lo)


def _prep_shards(inputs):
    import ml_dtypes

    ids = np.ascontiguousarray(np.asarray(inputs["input_ids"]).astype(np.int32))
    ee = np.asarray(inputs["entity_embeddings"], dtype=np.float32)
    mask = np.asarray(inputs["entity_mask"], dtype=np.float32)
    we = np.ascontiguousarray(np.asarray(inputs["word_embedding"], dtype=np.float32))
    W1 = np.asarray(inputs["W1"], dtype=np.float32)
    b1 = np.asarray(inputs["b1"], dtype=np.float32)
    W2 = np.asarray(inputs["W2"], dtype=np.float32)
    b2 = np.asarray(inputs["b2"], dtype=np.float32)

    w1_hi, w1_lo = _split_hl(W1)  # [KG, MH]
    w2_pad = np.concatenate([W2, np.zeros((KCH * 128 - MH, H), np.float32)], 0)
    w2p = w2_pad.reshape(KCH, 128, H).transpose(1, 0, 2).reshape(128, KCH * H)
    w2p_hi, w2p_lo = _split_hl(w2p)
    b2_hi, b2_lo = _split_hl(b2[None, :])
    b2hl = np.ascontiguousarray(np.concatenate([b2_hi, b2_lo], 0))  # [2, H]
    b1pad = np.concatenate([b1, np.zeros(KCH * 128 - MH, np.float32)])
    b1colT = np.ascontiguousarray(b1pad.reshape(KCH, 128).T)  # [128, KCH]

    in_maps = []
    for i in range(NCORES):
        sl = slice(BPC * i, BPC * (i + 1))
        ids_i = ids[sl].reshape(-1)  # [TOK]
        idsT = np.ascontiguousarray(ids_i.reshape(NCH, 128).T)  # [128, NCH]
        eeT = ee[sl].reshape(NE, KG).T  # [KG, NE]
        eeT_hi, eeT_lo = _split_hl(eeT)
        # block-diagonal [NE, TOK] mask, duplicated to [2*NE, TOK] so one
        # K=64 matmul covers both the hi and lo halves of entHL
        maskT = np.zeros((NE, TOK), np.float32)
        for b in range(BPC):
            maskT[b * E : (b + 1) * E, b * S : (b + 1) * S] = mask[BPC * i + b]
        maskT2 = np.ascontiguousarray(
            np.concatenate([maskT, maskT], 0).astype(ml_dtypes.bfloat16)
        )
        in_maps.append(
            {
                "idsT": idsT,
                "we": we,
                "eeT_hi": eeT_hi,
                "eeT_lo": eeT_lo,
                "w1_hi": w1_hi,
                "w1_lo": w1_lo,
                "b1row": np.ascontiguousarray(b1[None, :]),
                "b1colT": b1colT,
                "w2p_hi": w2p_hi,
                "w2p_lo": w2p_lo,
                "b2hl": b2hl,
                "maskT2": maskT2,
            }
        )
    return in_maps


def kernel(**inputs) -> np.ndarray:
    from concourse.bass_utils import run_bass_kernel_spmd

    trace = _maybe_enable_profiling()
    nc = _get_program()
    in_maps = _prep_shards(inputs)
    res = run_bass_kernel_spmd(
        nc, in_maps, core_ids=list(range(NCORES)), trace=trace
    )
    if trace and res.exec_time_ns is not None:
        print(f"HW exec time: {res.exec_time_ns} ns")
    out = np.concatenate(
        [res.results[i]["out"].reshape(BPC, S, H) for i in range(NCORES)], 0
    )
    return out


if __name__ == "__main__":
    rng = np.random.default_rng(0)
    inputs = {
        "input_ids": rng.integers(0, V, (B, S)).astype(np.int32),
        "entity_embeddings": rng.standard_normal((B, E, KG), dtype=np.float32),
        "entity_mask": (rng.random((B, E, S)) < 0.02).astype(np.float32),
        "word_embedding": rng.standard_normal((V, H), dtype=np.float32) * 0.02,
        "W1": rng.standard_normal((KG, MH), dtype=np.float32) * 0.02,
        "b1": np.zeros(MH, np.float32),
        "W2": rng.standard_normal((MH, H), dtype=np.float32) * 0.02,
        "b2": np.zeros(H, np.float32),
    }
    out = kernel(**inputs)
    ref = inputs["word_embedding"][inputs["input_ids"]] + np.einsum(
        "bes,beh->bsh",
        inputs["entity_mask"],
        np.maximum(
            inputs["entity_embeddings"] @ inputs["W1"] + inputs["b1"], 0.0
        )
        @ inputs["W2"]
        + inputs["b2"],
    )
    err = np.abs(out - ref).max() / max(np.abs(ref).max(), 1e-12)
    print("self-check rel err:", err)
